# revision 11
# baseline (speedup 1.0000x reference)
"""Causal self-attention (B=4, T=2048, C=1024, H=16) on 8 trn2 NeuronCores.

Sharding: tensor-parallel over heads — each core owns 2 heads (128 of the
1024 channel dims). Each core computes its Q/K/V slices from the full x,
runs causal attention for its heads over all batches, and produces a
partial output projection; the host sums the 8 partials (the all-reduce).

Layout trick: attention scores are computed transposed (S^T[tk, tq]) so
softmax needs no on-chip transposes anywhere in the inner loop:
  - S^T = kT.T @ qT              (kT/qT are [head_dim, tokens] in SBUF)
  - P = exp(S^T)                 (no row-max: scores ~ N(0,1), exp is safe)
  - out[tq, d] = P.T @ v_aug     (v_aug has a ones column -> denominator)
  - normalize with a per-partition scalar multiply (tq is the partition dim)
Causality = skip fully-invalid blocks + one 128x128 triangle mask multiply
on the diagonal block.
"""

import sys

if "/opt/trn_rl_repo" not in sys.path:
    sys.path.insert(0, "/opt/trn_rl_repo")

import ml_dtypes
import numpy as np

B, T, C, H = 4, 2048, 1024, 16
HD = C // H          # 64
NCORES = 8
HPC = H // NCORES    # heads per core = 2
DPC = HPC * HD       # channel dims per core = 128
N = B * T            # 8192 tokens
P = 128              # partitions
TCH = 512            # stage-1 token chunk (psum bank width, fp32)
KB = C // P          # k blocks in stage 1 = 8
NTCH = N // TCH      # 16
NQC = T // TCH       # tq chunks per (b,h) = 4
NTB = T // P         # 128-token blocks per batch = 16

BF16 = ml_dtypes.bfloat16

_CACHE = {}


def _build_nc():
    import concourse.tile as tile
    from concourse import bacc, mybir

    nc = bacc.Bacc(None, target_bir_lowering=False)
    f32 = mybir.dt.float32
    bf16 = mybir.dt.bfloat16
    AF = mybir.ActivationFunctionType

    # ---- DRAM I/O (per-core tensors; same program on all 8 cores) ----
    xt_d = nc.dram_tensor("xt", [C, N], bf16, kind="ExternalInput")
    wq_d = nc.dram_tensor("wq", [C, DPC], bf16, kind="ExternalInput")
    wk_d = nc.dram_tensor("wk", [C, DPC], bf16, kind="ExternalInput")
    wv_d = nc.dram_tensor("wv", [C, DPC], bf16, kind="ExternalInput")
    wp_d = nc.dram_tensor("wp", [DPC, C], bf16, kind="ExternalInput")
    bq_d = nc.dram_tensor("bq", [DPC, 1], f32, kind="ExternalInput")
    bk_d = nc.dram_tensor("bk", [DPC, 1], f32, kind="ExternalInput")
    bv_d = nc.dram_tensor("bv", [P, DPC], f32, kind="ExternalInput")
    tri_d = nc.dram_tensor("tri", [P, P], bf16, kind="ExternalInput")
    id_d = nc.dram_tensor("idn", [P, P], bf16, kind="ExternalInput")
    out_d = nc.dram_tensor("out", [N, C], f32, kind="ExternalOutput")

    with tile.TileContext(nc) as tc:
        with (
            tc.tile_pool(name="persist", bufs=1) as persist,
            tc.tile_pool(name="xp", bufs=12) as xp,
            tc.tile_pool(name="ptp", bufs=4) as ptp,
            tc.tile_pool(name="ysp", bufs=8) as ysp,
            tc.tile_pool(name="ytp", bufs=4) as ytp,
            tc.tile_pool(name="osp", bufs=3) as osp,
            tc.tile_pool(name="rcp", bufs=8) as rcp,
            tc.tile_pool(name="big", bufs=3, space="PSUM") as big,
            tc.tile_pool(name="avp", bufs=4, space="PSUM") as avp,
            tc.tile_pool(name="trp", bufs=1, space="PSUM") as trp,
        ):
            # ---- persistent SBUF ----
            qTs = persist.tile([P, N], bf16, tag="qTs")   # [dims, tokens]
            kTs = persist.tile([P, N], bf16, tag="kTs")
            # v blocks: per 128-token block: [v_h0 | 1 | v_h1 | 1] = 130 cols
            vs = persist.tile([P, (N // P) * 130], bf16, tag="vs")
            wqs = persist.tile([P, C], bf16, tag="wqs")   # 8 blocks of [128,128]
            wks = persist.tile([P, C], bf16, tag="wks")
            wvs = persist.tile([P, C], bf16, tag="wvs")
            wps = persist.tile([P, C], bf16, tag="wps")
            bqs = persist.tile([P, 1], f32, tag="bqs")
            bks = persist.tile([P, 1], f32, tag="bks")
            bvs = persist.tile([P, DPC], f32, tag="bvs")
            tri = persist.tile([P, P], bf16, tag="tri")
            idn = persist.tile([P, P], bf16, tag="idn")

            for k in range(KB):
                nc.sync.dma_start(out=wqs[:, k * P:(k + 1) * P],
                                  in_=wq_d[k * P:(k + 1) * P, :])
                nc.sync.dma_start(out=wks[:, k * P:(k + 1) * P],
                                  in_=wk_d[k * P:(k + 1) * P, :])
                nc.sync.dma_start(out=wvs[:, k * P:(k + 1) * P],
                                  in_=wv_d[k * P:(k + 1) * P, :])
            nc.sync.dma_start(out=wps[:, :], in_=wp_d[:, :])
            nc.sync.dma_start(out=bqs[:, :], in_=bq_d[:, :])
            nc.sync.dma_start(out=bks[:, :], in_=bk_d[:, :])
            nc.sync.dma_start(out=bvs[:, :], in_=bv_d[:, :])
            nc.sync.dma_start(out=tri[:, :], in_=tri_d[:, :])
            nc.sync.dma_start(out=idn[:, :], in_=id_d[:, :])

            # ones columns of v_aug (cols 64 and 129 of each 130-block)
            vs_r = vs.rearrange("p (t c) -> p t c", c=130)
            nc.vector.memset(vs_r[:, :, HD:HD + 1], 1.0)
            nc.vector.memset(vs_r[:, :, 2 * HD + 1:2 * HD + 2], 1.0)

            # ---- stage 1: qT, kT (dims x tokens) and v (tokens x dims) ----
            for tch in range(NTCH):
                t0 = tch * TCH
                xts = []
                for k in range(KB):
                    xtile = xp.tile([P, TCH], bf16, tag="xt")
                    nc.sync.dma_start(out=xtile,
                                      in_=xt_d[k * P:(k + 1) * P, t0:t0 + TCH])
                    xts.append(xtile)
                psq = big.tile([P, TCH], f32, tag="big")
                psk = big.tile([P, TCH], f32, tag="big")
                for k in range(KB):
                    nc.tensor.matmul(psq[:], wqs[:, k * P:(k + 1) * P], xts[k][:],
                                     start=(k == 0), stop=(k == KB - 1))
                    nc.tensor.matmul(psk[:], wks[:, k * P:(k + 1) * P], xts[k][:],
                                     start=(k == 0), stop=(k == KB - 1))
                nc.scalar.activation(qTs[:, t0:t0 + TCH], psq[:], AF.Identity,
                                     bias=bqs[:, :])
                nc.scalar.activation(kTs[:, t0:t0 + TCH], psk[:], AF.Identity,
                                     bias=bks[:, :])
                # v natural layout, accumulated per 128-token block
                for m in range(TCH // P):
                    tb = (t0 + m * P) // P
                    psv = avp.tile([P, P], f32, tag="av")
                    for k in range(KB):
                        nc.tensor.matmul(psv[:],
                                         xts[k][:, m * P:(m + 1) * P],
                                         wvs[:, k * P:(k + 1) * P],
                                         start=(k == 0), stop=(k == KB - 1))
                    for h in range(HPC):
                        nc.vector.tensor_add(
                            vs_r[:, tb, h * (HD + 1):h * (HD + 1) + HD],
                            psv[:, h * HD:(h + 1) * HD],
                            bvs[:, h * HD:(h + 1) * HD])

            # ---- stage 2: attention + projection, per batch ----
            for b in range(B):
                base = b * T
                ys_tiles = {}
                for c in range(NQC):
                    q0 = base + c * TCH
                    for m in range(NQC):
                        yt = ysp.tile([P, P], bf16, tag="ys")
                        ys_tiles[(c, m)] = yt
                    for h in range(HPC):
                        hq = h * HD
                        av_ps = [avp.tile([P, HD + 1], f32, tag="av",
                                          name=f"avps{m}")
                                 for m in range(NQC)]
                        for j in range(4 * c + 4):
                            r = j - 4 * c  # >=0 on the diagonal band
                            u0 = max(0, r) * P
                            st = big.tile([P, TCH], f32, tag="big")
                            nc.tensor.matmul(
                                st[:, u0:TCH],
                                kTs[hq:hq + HD, base + j * P:base + (j + 1) * P],
                                qTs[hq:hq + HD, q0 + u0:q0 + TCH],
                                start=True, stop=True)
                            pt = ptp.tile([P, TCH], bf16, tag="pt")
                            nc.scalar.activation(pt[:, u0:TCH], st[:, u0:TCH],
                                                 AF.Exp)
                            if r >= 0:
                                nc.vector.tensor_mul(pt[:, u0:u0 + P],
                                                     pt[:, u0:u0 + P], tri[:])
                            for m in range(NQC):
                                if j > 4 * c + m:
                                    continue
                                vtb = b * NTB + j
                                nc.tensor.matmul(
                                    av_ps[m][:],
                                    pt[:, m * P:(m + 1) * P],
                                    vs_r[:, vtb, h * (HD + 1):(h + 1) * (HD + 1)],
                                    start=(j == 0), stop=(j == 4 * c + m))
                        for m in range(NQC):
                            rec = rcp.tile([P, 1], f32, tag="rec")
                            nc.vector.reciprocal(rec[:], av_ps[m][:, HD:HD + 1])
                            nc.vector.tensor_scalar_mul(
                                ys_tiles[(c, m)][:, hq:hq + HD],
                                av_ps[m][:, 0:HD], rec[:])
                    # transpose y blocks and project as soon as a chunk is done
                    for m in range(NQC):
                        tb = c * 4 + m
                        tr_ps = trp.tile([P, P], bf16, tag="tr")
                        nc.tensor.transpose(tr_ps[:], ys_tiles[(c, m)][:], idn[:])
                        yt = ytp.tile([P, P], bf16, tag="yt")
                        nc.vector.tensor_copy(yt[:], tr_ps[:])
                        osb = osp.tile([P, C], f32, tag="os")
                        pp0 = big.tile([P, TCH], f32, tag="big")
                        nc.tensor.matmul(pp0[:], yt[:], wps[:, 0:TCH],
                                         start=True, stop=True)
                        nc.scalar.copy(osb[:, 0:TCH], pp0[:])
                        pp1 = big.tile([P, TCH], f32, tag="big")
                        nc.tensor.matmul(pp1[:], yt[:], wps[:, TCH:C],
                                         start=True, stop=True)
                        nc.vector.tensor_copy(osb[:, TCH:C], pp1[:])
                        row0 = base + tb * P
                        nc.sync.dma_start(out=out_d[row0:row0 + P, :], in_=osb[:])
    nc.compile()
    return nc


def _get_nc():
    if "nc" not in _CACHE:
        _CACHE["nc"] = _build_nc()
    return _CACHE["nc"]


def _make_in_maps(x, Wk, bk, Wq, bq, Wv, bv, Wp, bp):
    x2 = np.ascontiguousarray(x.reshape(N, C).T)          # [C, N] fp32
    xt = x2.astype(BF16)
    scale = 1.0 / np.sqrt(HD)
    wqt = (Wq.T * scale).astype(BF16)                     # [C, C]
    wkt = Wk.T.astype(BF16)
    wvt = Wv.T.astype(BF16)
    wpt = Wp.T.astype(BF16)                               # [C(in=y dims), C(out)]
    tri = np.triu(np.ones((P, P), np.float32)).astype(BF16)
    idn = np.eye(P, dtype=np.float32).astype(BF16)
    in_maps = []
    for cidx in range(NCORES):
        s = slice(cidx * DPC, (cidx + 1) * DPC)
        in_maps.append({
            "xt": xt,
            "wq": np.ascontiguousarray(wqt[:, s]),
            "wk": np.ascontiguousarray(wkt[:, s]),
            "wv": np.ascontiguousarray(wvt[:, s]),
            "wp": np.ascontiguousarray(wpt[s, :]),
            "bq": (bq[s] * scale).astype(np.float32).reshape(DPC, 1),
            "bk": bk[s].astype(np.float32).reshape(DPC, 1),
            "bv": np.ascontiguousarray(
                np.broadcast_to(bv[s].astype(np.float32), (P, DPC))),
            "tri": tri,
            "idn": idn,
        })
    return in_maps


def kernel(x, Wk, bk, Wq, bq, Wv, bv, Wp, bp):
    from concourse.bass_utils import run_bass_kernel_spmd

    nc = _get_nc()
    in_maps = _make_in_maps(x, Wk, bk, Wq, bq, Wv, bv, Wp, bp)
    res = run_bass_kernel_spmd(nc, in_maps, core_ids=list(range(NCORES)))
    acc = np.zeros((N, C), np.float64)
    for r in res.results:
        acc += r["out"].astype(np.float64)
    out = (acc + bp.astype(np.float64)).astype(np.float32)
    return out.reshape(B, T, C)
